# revision 14
# baseline (speedup 1.0000x reference)
"""ExllamaLinear (int4 GPTQ-style quantized linear) on 8 Trainium2 NeuronCores.

out = x @ dequant(qweight, qzeros, scales) + bias
  x: [4, 2048, 4096] fp16, qweight: [512, 11008] int32 (8x int4 nibbles along
  in_features), qzeros: [32, 1376] int32, scales: [32, 11008] fp16,
  bias: [11008] fp16, group_size 128.

Strategy: column-parallel over 8 cores (1376 out_features each), x replicated.
PE-bound problem: per-core fp16 matmul floor is 64 m-tiles x 32 k-tiles x
1376 cols = 2.818M PE cycles ~ 1174 us @2.4GHz (+ 2.2ns/instr hw decode =
~1188 us PE-busy floor). fp8 was measured numerically and rejected: pure
fp8 (one matmul) has 3.1% max rel err vs the 2e-2 budget, and even
single-side fp8 (2.1-2.3%) fails, so corrected variants need >=3 matmuls
and lose to fp16.

W is dequantized on the HOST (fp32 math, fp16 result), shipped k-major
[4096, 1376] per core. The kernel streams x^T and runs PSUM-accumulated
fp16 matmuls with the bias folded into the PSUM->SBUF drains.

Startup (the first ~75us) is supply-choreographed:
  - chunk 0 (tokens 0:512) runs 4 m-tiles per phase: phase A covers j0+j1
    (8 PSUM banks exactly), phase B covers j2 (4 banks). With 4 m-tiles
    per W k-tile the PE consumes W at only ~225GB/s, which the 3 DMA
    queues (sync/scalar HWDGE + gpsimd SWDGE, fanning out to 16 shared
    DMA engines at ~20GB/s each) can sustain -- so no k-split or fp16
    partial-drain machinery is needed at all.
  - k-tiles 0-1 are fetched as 128KB half-pieces for first-arrival
    latency (~4us faster than a 256KB piece on one engine); later W
    k-tiles are 256KB (j0j1) pieces, Wb (j2) and x arrive as 2-k-tile
    merged pieces to conserve dma_start sequencer triggers (~0.6us each).
  - dummy matmuls on a memset tile keep the PE p-state ramped (full
    clock needs 3us of continuous busy) and cover the DMA head.
  - phase tails run bank-major (per-bank k29..31 + stop + drain) so PSUM
    banks free up one-by-one just ahead of the next phase's consumption.
  - the A-phase output (columns 0:1024) is DMA'd per m-tile as soon as
    its two drains land; j2 columns follow after phase B.
Steady state (chunks 1..15) streams whole x^T k-tiles on the sync queue
(chunk 1 as pre-issued 2-k-tile merged pieces), 3 matmuls per (m,k), and
drains each m-tile in row-quarters on the scalar/gpsimd queues only (an
out trigger blocked on its drain must not stall x supply queued behind
it on sync). The final m-tile reorders its last k-step to (j2,j0,j1),
drains per-j and DMAs each j-piece immediately in row-split pieces
across all three queues to cut the serial tail.
"""
import sys

sys.path.insert(0, "/opt/trn_rl_repo")

import numpy as np

IN_F = 4096
OUT_F = 11008
P = 128
KT = IN_F // P           # 32 k-tiles
NCORES = 8
N = OUT_F // NCORES      # 1376 out features per core
M = 4 * 2048             # 8192 tokens
NJ = [(0, 512), (512, 512), (1024, 352)]   # n j-tiles (PSUM bank <= 512 fp32)
MCHUNK = 512             # x^T streaming chunk (tokens)
NWARM = 24               # dummy 128-col PE warm-up matmuls (~3.2us, ramps clock)
NFINE = 6                # k-tiles fetched as split half-pieces at the start
ATAIL = 3                # bank-major k-steps at each phase tail

_CACHE = {}


def _build_bass():
    import concourse.bass as bass
    import concourse.bacc as bacc
    import concourse.mybir as mybir
    import concourse.tile as tile
    import contextlib
    import itertools

    # Bacc (not plain Bass): its compile() splits multi-wait instructions via
    # InstEventSemaphore — TRN2 instructions encode at most 1 sync wait.
    nc = bacc.Bacc()
    # x arrives host-transposed (k-major): [IN_F, M]
    x = nc.dram_tensor("x", [IN_F, M], mybir.dt.float16, kind="ExternalInput")
    # W arrives host-dequantized fp16, k-major: [IN_F, N]
    w = nc.dram_tensor("w", [IN_F, N], mybir.dt.float16, kind="ExternalInput")
    bias = nc.dram_tensor("bias", [1, N], mybir.dt.float16,
                          kind="ExternalInput")
    out = nc.dram_tensor("out", [M, N], mybir.dt.float16,
                         kind="ExternalOutput")

    def t(h):
        return h.tensor if hasattr(h, "tensor") else h

    with tile.TileContext(nc) as tc:
        with contextlib.ExitStack() as ctx:
            wpool = ctx.enter_context(tc.tile_pool(name="w", bufs=1))
            x0p = ctx.enter_context(tc.tile_pool(name="x0", bufs=1))
            x1p = ctx.enter_context(tc.tile_pool(name="x1", bufs=16))
            xtp = ctx.enter_context(tc.tile_pool(name="xt", bufs=36))
            outp = ctx.enter_context(tc.tile_pool(name="out", bufs=2))
            psum = ctx.enter_context(tc.tile_pool(name="ps", bufs=8,
                                                  space="PSUM"))
            singles = ctx.enter_context(tc.tile_pool(name="singles", bufs=1))

            # --- PE warm-up: ramp the p-state and cover the DMA head.
            # The clock reaches full speed only after ~3us of CONTINUOUS
            # busy (and drops back after an idle gap), so the warm-up must
            # run gapless into the first real matmul. 128-col dummies keep
            # the memset (which gates the first matmul) short.
            dum = singles.tile([P, P], mybir.dt.float16)
            nc.vector.memset(dum, 0.0)
            scratch = psum.tile([P, 512], mybir.dt.float32, tag="ps",
                                name="scratch")
            for _ in range(NWARM):
                nc.tensor.matmul(scratch[:, 0:P], dum, dum,
                                 start=True, stop=True)

            # --- chunk-0 + W supply, issued in PE consumption order ---
            qs = itertools.cycle((nc.sync, nc.scalar, nc.gpsimd))
            w_tiles = [None] * KT       # [k] -> [AP per j]
            x0_tiles = [None] * KT      # [k] -> [P, MCHUNK] (chunk-0 x^T)

            # k-tiles 0..NFINE-1: three 128KB pieces each (two W halves +
            # one x tile), one per ring, so the early tiles land at the
            # ring cold-start cadence (~1.5us per 128KB item per ring)
            # instead of 256KB-piece latency. Their j2 (Wb) columns are
            # deferred to the Wb-pair block — phase B runs ~50us later.
            for k in range(NFINE):
                wa0 = wpool.tile([P, 512], mybir.dt.float16,
                                 tag=f"Wa{k}h0", name=f"Wa{k}h0")
                next(qs).dma_start(out=wa0, in_=w[k * P:(k + 1) * P, 0:512])
                wa1 = wpool.tile([P, 512], mybir.dt.float16,
                                 tag=f"Wa{k}h1", name=f"Wa{k}h1")
                next(qs).dma_start(out=wa1, in_=w[k * P:(k + 1) * P,
                                                  512:1024])
                w_tiles[k] = [wa0, wa1, None]
                xt = x0p.tile([P, MCHUNK], mybir.dt.float16,
                              tag=f"x0_{k}", name=f"x0_{k}")
                next(qs).dma_start(out=xt, in_=x[k * P:(k + 1) * P,
                                                 0:MCHUNK])
                x0_tiles[k] = xt

            # k-tiles NFINE..31: 256KB j0j1 pieces; x as 2-k-tile merged
            # 256KB pieces (fewer dma_start triggers — the sequencers cost
            # ~0.6us per trigger). Ring assignment is explicit and balanced
            # by measured ring bandwidth: the HWDGE rings (sync/scalar) hold
            # only 4 outstanding DMAs (~60-80GB/s each) while the gpsimd
            # SWDGE ring sustains ~180GB/s, so gpsimd carries every even Wa
            # plus a third of the x pairs (~97GB/s) and sync/scalar ~62GB/s.
            waoq = itertools.cycle((nc.sync, nc.scalar))
            xtpq = itertools.cycle((nc.sync, nc.scalar, nc.gpsimd))
            for k in range(NFINE, KT):
                wa = wpool.tile([P, 1024], mybir.dt.float16,
                                tag=f"Wa{k}", name=f"Wa{k}")
                waq = nc.gpsimd if k % 2 == 0 else next(waoq)
                waq.dma_start(out=wa, in_=w[k * P:(k + 1) * P, 0:1024])
                w_tiles[k] = [wa[:, 0:512], wa[:, 512:1024], None]
                if k % 2 == 1:
                    k0 = k - 1
                    xp = x0p.tile([P, 2 * MCHUNK], mybir.dt.float16,
                                  tag=f"x0p{k0}", name=f"x0p{k0}")
                    next(xtpq).dma_start(
                        out=xp,
                        in_=bass.AP(tensor=t(x), offset=(k0 * P) * M,
                                    ap=[[M, P], [P * M, 2], [1, MCHUNK]]))
                    x0_tiles[k0] = xp[:, 0:MCHUNK]
                    x0_tiles[k] = xp[:, MCHUNK:2 * MCHUNK]

            # Wb (j2 columns) as 2-k-tile merged 176KB pieces; phase B
            # doesn't start until ~60us in, so these can trail.
            for g in range(KT // 2):
                k0 = 2 * g
                wbp = wpool.tile([P, 2 * 352], mybir.dt.float16,
                                 tag=f"Wbp{g}", name=f"Wbp{g}")
                next(qs).dma_start(
                    out=wbp,
                    in_=bass.AP(tensor=t(w), offset=(k0 * P) * N + 1024,
                                ap=[[N, P], [P * N, 2], [1, 352]]))
                w_tiles[k0][2] = wbp[:, 0:352]
                w_tiles[k0 + 1][2] = wbp[:, 352:704]

            # bias broadcast across partitions (fp16, DVE upconverts on the
            # drain add); first needed by the phase-A drains (~55us).
            bias_b = singles.tile([P, N], mybir.dt.float16)
            biasq = itertools.cycle((nc.scalar, nc.gpsimd))
            for noff, nsz in ((0, 344), (344, 344), (688, 344), (1032, 344)):
                next(biasq).dma_start(
                    out=bias_b[:, noff:noff + nsz],
                    in_=bass.AP(tensor=t(bias), offset=noff,
                                ap=[[0, P], [1, nsz]]),
                )

            # chunk-1 x as 2-k-tile merged 256KB pieces split across the
            # sync+scalar rings (sync alone lands the last pairs ~5us late),
            # queued behind all of chunk-0's supply (needed from ~80us).
            xt1_tiles = [None] * KT
            x1q = itertools.cycle((nc.sync, nc.scalar))
            for g in range(KT // 2):
                k0 = 2 * g
                xp = x1p.tile([P, 2 * MCHUNK], mybir.dt.float16, tag="xT1",
                              name=f"x1p{g}")
                next(x1q).dma_start(
                    out=xp,
                    in_=bass.AP(tensor=t(x), offset=(k0 * P) * M + MCHUNK,
                                ap=[[M, P], [P * M, 2], [1, MCHUNK]]))
                xt1_tiles[k0] = xp[:, 0:MCHUNK]
                xt1_tiles[k0 + 1] = xp[:, MCHUNK:2 * MCHUNK]

            # out triggers must NOT ride the sync queue mid-stream: a
            # blocked out trigger (waiting on its drain) would stall every
            # later x-tile supply DMA queued behind it. sync = input supply.
            outq = (nc.scalar, nc.gpsimd, nc.scalar, nc.gpsimd)

            def drain(ps_list, c, mt):
                ot = outp.tile([P, N], mybir.dt.float16, tag="ot",
                               name=f"ot{c}_{mt}")
                for j, (noff, nsz) in enumerate(NJ):
                    nc.vector.tensor_tensor(
                        ot[:, noff:noff + nsz],
                        ps_list[j],
                        bias_b[:, noff:noff + nsz],
                        mybir.AluOpType.add,
                    )
                m0 = c * MCHUNK + mt * P
                # quarter the out DMA so the final tile has no serial tail
                for qi, q in enumerate(outq):
                    p0 = qi * (P // 4)
                    p1 = p0 + P // 4
                    q.dma_start(out=out[m0 + p0:m0 + p1, :],
                                in_=ot[p0:p1, :])

            # --- chunk 0: 4 m-tiles per phase. Phase A = j0+j1 (8 PSUM
            # banks), phase B = j2 (4 banks). Phase tails run bank-major
            # with interleaved drains so banks free up ahead of the next
            # phase; the A output (cols 0:1024) ships per m-tile early.
            AK = KT - ATAIL
            abanks = [(mt, j) for mt in range(4) for j in range(2)]
            psA = {}
            for mt, j in abanks:
                ps_full = psum.tile([P, 512], mybir.dt.float32, tag="ps",
                                    name=f"psA{mt}_{j}")
                psA[(mt, j)] = ps_full

            def mmA(k, mt, j):
                lhsT = x0_tiles[k][:, mt * P:(mt + 1) * P]
                nc.tensor.matmul(psA[(mt, j)], lhsT, w_tiles[k][j],
                                 start=(k == 0), stop=(k == KT - 1))

            for k in range(AK):
                for mt, j in abanks:
                    mmA(k, mt, j)
            for mt, j in abanks:
                for k in range(AK, KT):
                    mmA(k, mt, j)
            # drains + per-m-tile early out DMA for columns 0:1024
            otA = {}
            for mt, j in abanks:
                noff, nsz = NJ[j]
                if j == 0:
                    otA[mt] = outp.tile([P, 1024], mybir.dt.float16,
                                        tag="otA", name=f"otA{mt}")
                nc.vector.tensor_tensor(
                    otA[mt][:, noff:noff + nsz], psA[(mt, j)],
                    bias_b[:, noff:noff + nsz], mybir.AluOpType.add)
                if j == 1:
                    m0 = mt * P
                    for qi, q in enumerate((nc.scalar, nc.gpsimd)):
                        p0 = qi * (P // 2)
                        p1 = p0 + P // 2
                        q.dma_start(out=out[m0 + p0:m0 + p1, 0:1024],
                                    in_=otA[mt][p0:p1, :])

            # phase B: j2, 4 banks (ring slots freed by the first A drains)
            psB = {}
            for mt in range(4):
                ps_full = psum.tile([P, 512], mybir.dt.float32, tag="ps",
                                    name=f"psB{mt}")
                psB[mt] = ps_full[:, 0:352]

            def mmB(k, mt):
                lhsT = x0_tiles[k][:, mt * P:(mt + 1) * P]
                nc.tensor.matmul(psB[mt], lhsT, w_tiles[k][2],
                                 start=(k == 0), stop=(k == KT - 1))

            for k in range(AK):
                for mt in range(4):
                    mmB(k, mt)
            b2q = itertools.cycle((nc.scalar, nc.gpsimd))
            for mt in range(4):
                for k in range(AK, KT):
                    mmB(k, mt)
                otB = outp.tile([P, 352], mybir.dt.float16,
                                tag="otB", name=f"otB{mt}")
                nc.vector.tensor_tensor(otB, psB[mt], bias_b[:, 1024:1376],
                                        mybir.AluOpType.add)
                m0 = mt * P
                next(b2q).dma_start(out=out[m0:m0 + P, 1024:1376], in_=otB)

            # --- chunks 1..15: stream whole x^T tiles, one m-tile at a time
            for c in range(1, M // MCHUNK):
                m_base = c * MCHUNK
                if c == 1:
                    xt_tiles = xt1_tiles
                else:
                    xt_tiles = []
                    for i in range(KT):
                        xt = xtp.tile([P, MCHUNK], mybir.dt.float16,
                                      tag="xT", name=f"xt{c}_{i}")
                        nc.sync.dma_start(
                            out=xt,
                            in_=x[i * P:(i + 1) * P, m_base:m_base + MCHUNK],
                        )
                        xt_tiles.append(xt)

                for mt in range(MCHUNK // P):
                    last = (c == M // MCHUNK - 1) and (mt == MCHUNK // P - 1)
                    ps_list = []
                    for j, (_, nsz) in enumerate(NJ):
                        ps_full = psum.tile([P, 512], mybir.dt.float32,
                                            tag="ps", name=f"ps{c}_{mt}_{j}")
                        ps_list.append(ps_full[:, :nsz])

                    def mm(i, j):
                        lhsT = xt_tiles[i][:, mt * P:(mt + 1) * P]
                        nc.tensor.matmul(
                            ps_list[j], lhsT, w_tiles[i][j],
                            start=(i == 0), stop=(i == KT - 1))

                    ilast = KT if not last else KT - ATAIL
                    for i in range(ilast):
                        for j in range(len(NJ)):
                            mm(i, j)
                    if not last:
                        drain(ps_list, c, mt)
                    else:
                        # final m-tile: stagger the last k-steps per j so
                        # each bank stops, drains, and ships while the next
                        # j's matmuls still run; j2 (smallest piece) last.
                        ot = outp.tile([P, N], mybir.dt.float16, tag="ot",
                                       name="ot_final")
                        m0 = c * MCHUNK + mt * P
                        for j in range(len(NJ)):
                            noff, nsz = NJ[j]
                            for i in range(KT - ATAIL, KT):
                                mm(i, j)
                            nc.vector.tensor_tensor(
                                ot[:, noff:noff + nsz], ps_list[j],
                                bias_b[:, noff:noff + nsz],
                                mybir.AluOpType.add)
                            for qi, q in enumerate((nc.sync, nc.scalar)):
                                p0 = qi * (P // 2)
                                p1 = p0 + P // 2
                                q.dma_start(
                                    out=out[m0 + p0:m0 + p1,
                                            noff:noff + nsz],
                                    in_=ot[p0:p1, noff:noff + nsz])
    nc.compile()
    return nc


def _get_nc():
    if "nc" not in _CACHE:
        _CACHE["nc"] = _build_bass()
    return _CACHE["nc"]


def _prep_inputs(x, qweight, qzeros, scales, bias):
    """Host-side sharding + layout prep. Returns per-core in_maps."""
    x = np.ascontiguousarray(np.asarray(x)).reshape(M, IN_F)
    qweight = np.asarray(qweight)
    qzeros = np.asarray(qzeros)
    scales_np = np.asarray(scales)
    bias_np = np.asarray(bias)

    # transpose x to k-major — the device then needs no transposes at all
    x_dev = np.ascontiguousarray(x.T)

    # host dequant (fp32 math, fp16 result), same convention as the
    # reference: w = (q - (z + 1)) * scale per 128-row group
    sh = (np.arange(8, dtype=np.int32) * 4)
    w_int = ((qweight[:, None, :] >> sh[None, :, None]) & 15).reshape(
        IN_F, OUT_F)
    z_int = ((qzeros[:, :, None] >> sh[None, None, :]) & 15).reshape(
        KT, OUT_F)
    W = ((w_int.reshape(KT, P, OUT_F).astype(np.float32)
          - (z_int + 1).astype(np.float32)[:, None, :])
         * scales_np.astype(np.float32)[:, None, :]
         ).reshape(IN_F, OUT_F).astype(np.float16)

    in_maps = []
    for cid in range(NCORES):
        sl = slice(cid * N, (cid + 1) * N)
        in_maps.append({
            "x": x_dev,
            "w": np.ascontiguousarray(W[:, sl]),
            "bias": np.ascontiguousarray(bias_np[sl]).reshape(1, N),
            })
    return in_maps


def _run(in_maps, trace=False):
    from concourse.bass_utils import run_bass_kernel_spmd
    nc = _get_nc()
    return run_bass_kernel_spmd(nc, in_maps, core_ids=list(range(NCORES)),
                                trace=trace)


def kernel(x, qweight, qzeros, scales, bias):
    in_maps = _prep_inputs(x, qweight, qzeros, scales, bias)
    res = _run(in_maps, trace=False)
    out = np.concatenate([r["out"] for r in res.results], axis=1)
    return out.reshape(4, 2048, OUT_F)


# revision 16
# speedup vs baseline: 1.0027x; 1.0027x over previous
"""ExllamaLinear (int4 GPTQ-style quantized linear) on 8 Trainium2 NeuronCores.

out = x @ dequant(qweight, qzeros, scales) + bias
  x: [4, 2048, 4096] fp16, qweight: [512, 11008] int32 (8x int4 nibbles along
  in_features), qzeros: [32, 1376] int32, scales: [32, 11008] fp16,
  bias: [11008] fp16, group_size 128.

Strategy: column-parallel over 8 cores (1376 out_features each), x replicated.
PE-bound problem: per-core fp16 matmul floor is 64 m-tiles x 32 k-tiles x
1376 cols = 2.818M PE cycles ~ 1174 us @2.4GHz (+ 2.2ns/instr hw decode =
~1188 us PE-busy floor). fp8 was measured numerically and rejected: pure
fp8 (one matmul) has 3.1% max rel err vs the 2e-2 budget, and even
single-side fp8 (2.1-2.3%) fails, so corrected variants need >=3 matmuls
and lose to fp16.

W is dequantized on the HOST (fp32 math, fp16 result), shipped k-major
[4096, 1376] per core. The kernel streams x^T and runs PSUM-accumulated
fp16 matmuls with the bias folded into the PSUM->SBUF drains.

Startup (the first ~75us) is supply-choreographed:
  - chunk 0 (tokens 0:512) runs 4 m-tiles per phase: phase A covers j0+j1
    (8 PSUM banks exactly), phase B covers j2 (4 banks). With 4 m-tiles
    per W k-tile the PE consumes W at only ~225GB/s, which the 3 DMA
    queues (sync/scalar HWDGE + gpsimd SWDGE, fanning out to 16 shared
    DMA engines at ~20GB/s each) can sustain -- so no k-split or fp16
    partial-drain machinery is needed at all.
  - k-tiles 0-1 are fetched as 128KB half-pieces for first-arrival
    latency (~4us faster than a 256KB piece on one engine); later W
    k-tiles are 256KB (j0j1) pieces, Wb (j2) and x arrive as 2-k-tile
    merged pieces to conserve dma_start sequencer triggers (~0.6us each).
  - dummy matmuls on a memset tile keep the PE p-state ramped (full
    clock needs 3us of continuous busy) and cover the DMA head.
  - phase tails run bank-major (per-bank k29..31 + stop + drain) so PSUM
    banks free up one-by-one just ahead of the next phase's consumption.
  - the A-phase output (columns 0:1024) is DMA'd per m-tile as soon as
    its two drains land; j2 columns follow after phase B.
Steady state (chunks 1..15) streams whole x^T k-tiles on the sync queue
(chunk 1 as pre-issued 2-k-tile merged pieces), 3 matmuls per (m,k), and
drains each m-tile in row-quarters on the scalar/gpsimd queues only (an
out trigger blocked on its drain must not stall x supply queued behind
it on sync). The final m-tile reorders its last k-step to (j2,j0,j1),
drains per-j and DMAs each j-piece immediately in row-split pieces
across all three queues to cut the serial tail.
"""
import sys

sys.path.insert(0, "/opt/trn_rl_repo")

import numpy as np

IN_F = 4096
OUT_F = 11008
P = 128
KT = IN_F // P           # 32 k-tiles
NCORES = 8
N = OUT_F // NCORES      # 1376 out features per core
M = 4 * 2048             # 8192 tokens
NJ = [(0, 512), (512, 512), (1024, 352)]   # n j-tiles (PSUM bank <= 512 fp32)
MCHUNK = 512             # x^T streaming chunk (tokens)
NWARM = 28               # dummy 128-col PE warm-up matmuls (~3us, ramps clock)
NFINE = 4                # k-tiles fetched as split half-pieces at the start
NSINGLE = 12             # k-tiles whose x arrives as 128KB singles (then pairs)
ATAIL = 3                # bank-major k-steps at each phase tail

_CACHE = {}


def _build_bass():
    import concourse.bass as bass
    import concourse.bacc as bacc
    import concourse.mybir as mybir
    import concourse.tile as tile
    import contextlib
    import itertools

    # Bacc (not plain Bass): its compile() splits multi-wait instructions via
    # InstEventSemaphore — TRN2 instructions encode at most 1 sync wait.
    nc = bacc.Bacc()
    # x arrives host-transposed (k-major): [IN_F, M]
    x = nc.dram_tensor("x", [IN_F, M], mybir.dt.float16, kind="ExternalInput")
    # W arrives host-dequantized fp16, k-major: [IN_F, N]
    w = nc.dram_tensor("w", [IN_F, N], mybir.dt.float16, kind="ExternalInput")
    bias = nc.dram_tensor("bias", [1, N], mybir.dt.float16,
                          kind="ExternalInput")
    out = nc.dram_tensor("out", [M, N], mybir.dt.float16,
                         kind="ExternalOutput")

    def t(h):
        return h.tensor if hasattr(h, "tensor") else h

    with tile.TileContext(nc) as tc:
        with contextlib.ExitStack() as ctx:
            wpool = ctx.enter_context(tc.tile_pool(name="w", bufs=1))
            x0p = ctx.enter_context(tc.tile_pool(name="x0", bufs=1))
            x1p = ctx.enter_context(tc.tile_pool(name="x1", bufs=16))
            xtp = ctx.enter_context(tc.tile_pool(name="xt", bufs=36))
            outp = ctx.enter_context(tc.tile_pool(name="out", bufs=2))
            psum = ctx.enter_context(tc.tile_pool(name="ps", bufs=8,
                                                  space="PSUM"))
            singles = ctx.enter_context(tc.tile_pool(name="singles", bufs=1))

            # --- PE warm-up: ramp the p-state and cover the DMA head.
            # The clock reaches full speed only after ~3us of CONTINUOUS
            # busy (and drops back after an idle gap), so the warm-up must
            # run gapless into the first real matmul. 128-col dummies keep
            # the memset (which gates the first matmul) short.
            dum = singles.tile([P, P], mybir.dt.float16)
            nc.vector.memset(dum, 0.0)
            scratch = psum.tile([P, 512], mybir.dt.float32, tag="ps",
                                name="scratch")
            for _ in range(NWARM):
                nc.tensor.matmul(scratch[:, 0:P], dum, dum,
                                 start=True, stop=True)

            # --- chunk-0 + W supply, issued in PE consumption order ---
            # Ring dispatch is weighted-fair-queued by bytes: the HWDGE
            # rings (sync/scalar) hold only 4 outstanding DMAs (~78GB/s
            # each) while the gpsimd SWDGE ring sustains ~170GB/s, so each
            # piece goes to the ring with the earliest virtual finish time.
            rings = [(nc.sync, 1.0), (nc.scalar, 1.0), (nc.gpsimd, 2.1)]
            loads = [0.0, 0.0, 0.0]

            def issue(nbytes, out, in_):
                i = min(range(3),
                        key=lambda r: (loads[r] + nbytes) / rings[r][1])
                loads[i] += nbytes
                rings[i][0].dma_start(out=out, in_=in_)

            w_tiles = [None] * KT       # [k] -> [AP per j]
            x0_tiles = [None] * KT      # [k] -> [P, MCHUNK] (chunk-0 x^T)

            # k-tiles 0..NFINE-1: three 128KB pieces each (two W halves +
            # one x tile), spread across rings, so the early tiles land at
            # the ring cold-start cadence (~1.5us per 128KB item per ring)
            # instead of 256KB-piece latency. Their j2 (Wb) columns are
            # deferred to the Wb-pair block — phase B runs ~50us later.
            for k in range(NFINE):
                wa0 = wpool.tile([P, 512], mybir.dt.float16,
                                 tag=f"Wa{k}h0", name=f"Wa{k}h0")
                issue(131072, wa0, w[k * P:(k + 1) * P, 0:512])
                wa1 = wpool.tile([P, 512], mybir.dt.float16,
                                 tag=f"Wa{k}h1", name=f"Wa{k}h1")
                issue(131072, wa1, w[k * P:(k + 1) * P, 512:1024])
                w_tiles[k] = [wa0, wa1, None]
                xt = x0p.tile([P, MCHUNK], mybir.dt.float16,
                              tag=f"x0_{k}", name=f"x0_{k}")
                issue(131072, xt, x[k * P:(k + 1) * P, 0:MCHUNK])
                x0_tiles[k] = xt

            # k-tiles NFINE..31: 256KB j0j1 W pieces; x as 128KB singles
            # through the transition region (a ring item-size jump right
            # behind the fine pieces starves the PE), then 2-k-tile merged
            # 256KB pairs to conserve dma_start triggers (~0.6us each).
            for k in range(NFINE, KT):
                wa = wpool.tile([P, 1024], mybir.dt.float16,
                                tag=f"Wa{k}", name=f"Wa{k}")
                issue(262144, wa, w[k * P:(k + 1) * P, 0:1024])
                w_tiles[k] = [wa[:, 0:512], wa[:, 512:1024], None]
                if k < NSINGLE:
                    xt = x0p.tile([P, MCHUNK], mybir.dt.float16,
                                  tag=f"x0_{k}", name=f"x0_{k}")
                    issue(131072, xt, x[k * P:(k + 1) * P, 0:MCHUNK])
                    x0_tiles[k] = xt
                elif k % 2 == 1:
                    k0 = k - 1
                    xp = x0p.tile([P, 2 * MCHUNK], mybir.dt.float16,
                                  tag=f"x0p{k0}", name=f"x0p{k0}")
                    issue(262144, xp,
                          bass.AP(tensor=t(x), offset=(k0 * P) * M,
                                  ap=[[M, P], [P * M, 2], [1, MCHUNK]]))
                    x0_tiles[k0] = xp[:, 0:MCHUNK]
                    x0_tiles[k] = xp[:, MCHUNK:2 * MCHUNK]

            # Wb (j2 columns) as 2-k-tile merged 176KB pieces; phase B
            # doesn't start until ~60us in, so these can trail.
            for g in range(KT // 2):
                k0 = 2 * g
                wbp = wpool.tile([P, 2 * 352], mybir.dt.float16,
                                 tag=f"Wbp{g}", name=f"Wbp{g}")
                issue(180224, wbp,
                      bass.AP(tensor=t(w), offset=(k0 * P) * N + 1024,
                              ap=[[N, P], [P * N, 2], [1, 352]]))
                w_tiles[k0][2] = wbp[:, 0:352]
                w_tiles[k0 + 1][2] = wbp[:, 352:704]

            # bias broadcast across partitions (fp16, DVE upconverts on the
            # drain add); first needed by the phase-A drains (~55us).
            bias_b = singles.tile([P, N], mybir.dt.float16)
            for noff, nsz in ((0, 344), (344, 344), (688, 344), (1032, 344)):
                issue(88064, bias_b[:, noff:noff + nsz],
                      bass.AP(tensor=t(bias), offset=noff,
                              ap=[[0, P], [1, nsz]]))

            # chunk-1 x as 2-k-tile merged 256KB pieces, queued behind all
            # of chunk-0's supply (needed from ~80us).
            xt1_tiles = [None] * KT
            for g in range(KT // 2):
                k0 = 2 * g
                xp = x1p.tile([P, 2 * MCHUNK], mybir.dt.float16, tag="xT1",
                              name=f"x1p{g}")
                issue(262144, xp,
                      bass.AP(tensor=t(x), offset=(k0 * P) * M + MCHUNK,
                              ap=[[M, P], [P * M, 2], [1, MCHUNK]]))
                xt1_tiles[k0] = xp[:, 0:MCHUNK]
                xt1_tiles[k0 + 1] = xp[:, MCHUNK:2 * MCHUNK]

            # out triggers must NOT ride the sync queue mid-stream: a
            # blocked out trigger (waiting on its drain) would stall every
            # later x-tile supply DMA queued behind it. sync = input supply.
            outq = (nc.scalar, nc.gpsimd, nc.scalar, nc.gpsimd)

            def drain(ps_list, c, mt):
                ot = outp.tile([P, N], mybir.dt.float16, tag="ot",
                               name=f"ot{c}_{mt}")
                for j, (noff, nsz) in enumerate(NJ):
                    nc.vector.tensor_tensor(
                        ot[:, noff:noff + nsz],
                        ps_list[j],
                        bias_b[:, noff:noff + nsz],
                        mybir.AluOpType.add,
                    )
                m0 = c * MCHUNK + mt * P
                # quarter the out DMA so the final tile has no serial tail
                for qi, q in enumerate(outq):
                    p0 = qi * (P // 4)
                    p1 = p0 + P // 4
                    q.dma_start(out=out[m0 + p0:m0 + p1, :],
                                in_=ot[p0:p1, :])

            # --- chunk 0: 4 m-tiles per phase. Phase A = j0+j1 (8 PSUM
            # banks), phase B = j2 (4 banks). Phase tails run bank-major
            # with interleaved drains so banks free up ahead of the next
            # phase; the A output (cols 0:1024) ships per m-tile early.
            AK = KT - ATAIL
            abanks = [(mt, j) for mt in range(4) for j in range(2)]
            psA = {}
            for mt, j in abanks:
                ps_full = psum.tile([P, 512], mybir.dt.float32, tag="ps",
                                    name=f"psA{mt}_{j}")
                psA[(mt, j)] = ps_full

            def mmA(k, mt, j):
                lhsT = x0_tiles[k][:, mt * P:(mt + 1) * P]
                nc.tensor.matmul(psA[(mt, j)], lhsT, w_tiles[k][j],
                                 start=(k == 0), stop=(k == KT - 1))

            for k in range(AK):
                for mt, j in abanks:
                    mmA(k, mt, j)
            for mt, j in abanks:
                for k in range(AK, KT):
                    mmA(k, mt, j)
            # drains + per-m-tile early out DMA for columns 0:1024
            otA = {}
            for mt, j in abanks:
                noff, nsz = NJ[j]
                if j == 0:
                    otA[mt] = outp.tile([P, 1024], mybir.dt.float16,
                                        tag="otA", name=f"otA{mt}")
                nc.vector.tensor_tensor(
                    otA[mt][:, noff:noff + nsz], psA[(mt, j)],
                    bias_b[:, noff:noff + nsz], mybir.AluOpType.add)
                if j == 1:
                    m0 = mt * P
                    for qi, q in enumerate((nc.scalar, nc.gpsimd)):
                        p0 = qi * (P // 2)
                        p1 = p0 + P // 2
                        q.dma_start(out=out[m0 + p0:m0 + p1, 0:1024],
                                    in_=otA[mt][p0:p1, :])

            # phase B: j2, 4 banks (ring slots freed by the first A drains)
            psB = {}
            for mt in range(4):
                ps_full = psum.tile([P, 512], mybir.dt.float32, tag="ps",
                                    name=f"psB{mt}")
                psB[mt] = ps_full[:, 0:352]

            def mmB(k, mt):
                lhsT = x0_tiles[k][:, mt * P:(mt + 1) * P]
                nc.tensor.matmul(psB[mt], lhsT, w_tiles[k][2],
                                 start=(k == 0), stop=(k == KT - 1))

            for k in range(AK):
                for mt in range(4):
                    mmB(k, mt)
            b2q = itertools.cycle((nc.scalar, nc.gpsimd))
            for mt in range(4):
                for k in range(AK, KT):
                    mmB(k, mt)
                otB = outp.tile([P, 352], mybir.dt.float16,
                                tag="otB", name=f"otB{mt}")
                nc.vector.tensor_tensor(otB, psB[mt], bias_b[:, 1024:1376],
                                        mybir.AluOpType.add)
                m0 = mt * P
                next(b2q).dma_start(out=out[m0:m0 + P, 1024:1376], in_=otB)

            # --- chunks 1..15: stream whole x^T tiles, one m-tile at a time
            for c in range(1, M // MCHUNK):
                m_base = c * MCHUNK
                if c == 1:
                    xt_tiles = xt1_tiles
                else:
                    xt_tiles = []
                    for i in range(KT):
                        xt = xtp.tile([P, MCHUNK], mybir.dt.float16,
                                      tag="xT", name=f"xt{c}_{i}")
                        nc.sync.dma_start(
                            out=xt,
                            in_=x[i * P:(i + 1) * P, m_base:m_base + MCHUNK],
                        )
                        xt_tiles.append(xt)

                for mt in range(MCHUNK // P):
                    last = (c == M // MCHUNK - 1) and (mt == MCHUNK // P - 1)
                    ps_list = []
                    for j, (_, nsz) in enumerate(NJ):
                        ps_full = psum.tile([P, 512], mybir.dt.float32,
                                            tag="ps", name=f"ps{c}_{mt}_{j}")
                        ps_list.append(ps_full[:, :nsz])

                    def mm(i, j):
                        lhsT = xt_tiles[i][:, mt * P:(mt + 1) * P]
                        nc.tensor.matmul(
                            ps_list[j], lhsT, w_tiles[i][j],
                            start=(i == 0), stop=(i == KT - 1))

                    ilast = KT if not last else KT - ATAIL
                    for i in range(ilast):
                        for j in range(len(NJ)):
                            mm(i, j)
                    if not last:
                        drain(ps_list, c, mt)
                    else:
                        # final m-tile: stagger the last k-steps per j so
                        # each bank stops, drains, and ships while the next
                        # j's matmuls still run; j2 (smallest piece) last.
                        ot = outp.tile([P, N], mybir.dt.float16, tag="ot",
                                       name="ot_final")
                        m0 = c * MCHUNK + mt * P
                        for j in range(len(NJ)):
                            noff, nsz = NJ[j]
                            for i in range(KT - ATAIL, KT):
                                mm(i, j)
                            nc.vector.tensor_tensor(
                                ot[:, noff:noff + nsz], ps_list[j],
                                bias_b[:, noff:noff + nsz],
                                mybir.AluOpType.add)
                            for qi, q in enumerate((nc.sync, nc.scalar)):
                                p0 = qi * (P // 2)
                                p1 = p0 + P // 2
                                q.dma_start(
                                    out=out[m0 + p0:m0 + p1,
                                            noff:noff + nsz],
                                    in_=ot[p0:p1, noff:noff + nsz])
    nc.compile()
    return nc


def _get_nc():
    if "nc" not in _CACHE:
        _CACHE["nc"] = _build_bass()
    return _CACHE["nc"]


def _prep_inputs(x, qweight, qzeros, scales, bias):
    """Host-side sharding + layout prep. Returns per-core in_maps."""
    x = np.ascontiguousarray(np.asarray(x)).reshape(M, IN_F)
    qweight = np.asarray(qweight)
    qzeros = np.asarray(qzeros)
    scales_np = np.asarray(scales)
    bias_np = np.asarray(bias)

    # transpose x to k-major — the device then needs no transposes at all
    x_dev = np.ascontiguousarray(x.T)

    # host dequant (fp32 math, fp16 result), same convention as the
    # reference: w = (q - (z + 1)) * scale per 128-row group
    sh = (np.arange(8, dtype=np.int32) * 4)
    w_int = ((qweight[:, None, :] >> sh[None, :, None]) & 15).reshape(
        IN_F, OUT_F)
    z_int = ((qzeros[:, :, None] >> sh[None, None, :]) & 15).reshape(
        KT, OUT_F)
    W = ((w_int.reshape(KT, P, OUT_F).astype(np.float32)
          - (z_int + 1).astype(np.float32)[:, None, :])
         * scales_np.astype(np.float32)[:, None, :]
         ).reshape(IN_F, OUT_F).astype(np.float16)

    in_maps = []
    for cid in range(NCORES):
        sl = slice(cid * N, (cid + 1) * N)
        in_maps.append({
            "x": x_dev,
            "w": np.ascontiguousarray(W[:, sl]),
            "bias": np.ascontiguousarray(bias_np[sl]).reshape(1, N),
            })
    return in_maps


def _run(in_maps, trace=False):
    from concourse.bass_utils import run_bass_kernel_spmd
    nc = _get_nc()
    return run_bass_kernel_spmd(nc, in_maps, core_ids=list(range(NCORES)),
                                trace=trace)


def kernel(x, qweight, qzeros, scales, bias):
    in_maps = _prep_inputs(x, qweight, qzeros, scales, bias)
    res = _run(in_maps, trace=False)
    out = np.concatenate([r["out"] for r in res.results], axis=1)
    return out.reshape(4, 2048, OUT_F)
